# revision 1
# baseline (speedup 1.0000x reference)
"""MultiHeadAttention Trainium2 kernel (B=4, T=2048, C=1024, H=16, D=64).

Sharding: 8 cores = 4 batches x 2 head-groups. Core c handles batch c//2 and
heads (c%2)*8 .. (c%2)*8+7. Each core computes QKV projections for its head
group, attention, and a partial output projection (its head group's rows of
Wp). Host sums the two partials per batch and adds bp.

Device layout notes:
- All matmul operands are float32r (TF32-like, full PE rate at N>=512,
  ~1.5e-4 matmul relative error).
- x arrives from host pre-transposed AND pre-tiled as xT [CI, QB, 128, 512]
  so every [128,512] chunk DMA is fully contiguous.
- Q,K are computed transposed (QT/KT [c_out, T]); scores are computed
  transposed (scoresT [k, q]) so softmax exp runs on ACT and the AV matmul
  (lhsT=V_ext [k,65], rhs=PT [k,q]) directly yields att_outT [65, q] with
  row 64 = softmax denominator via a ones-column in V_ext. V_ext stays
  resident in SBUF.
- QK matmuls for the two heads of a partition-tile pair are packed into PE
  row halves (tile_position via base_partition) for 2x throughput at K=64.
- AV matmuls are interleaved into the QK/exp group loop so the probability
  tiles are consumed as soon as ACT produces them.
"""
import sys
sys.path.insert(0, '/opt/trn_rl_repo')
from contextlib import ExitStack

import numpy as np

import concourse.bass as bass
import concourse.tile as tile
from concourse import mybir, bacc
from concourse.bass_utils import run_bass_kernel_spmd

F32 = mybir.dt.float32
F32R = mybir.dt.float32r


def _patch_compile(nc):
    """Bacc.compile leaves multi-wait instructions that walrus rejects
    (fuse passes re-merge event-semaphore waits after the last
    generate_event_semaphores). Re-split after every compile."""
    import bass_rust
    orig = nc.compile

    def patched():
        orig()
        bass_rust.generate_event_semaphores(nc)

    nc.compile = patched


B, T, C = 4, 2048, 1024
H = 16
D = C // H            # 64
HL = 8                # heads per core
CL = HL * D           # 512 local channels
PAIRS = HL // 2       # head pairs (one 128-partition tile each)
CI = C // 128         # 8 contraction tiles over C
TT = T // 128         # 16 token tiles
QB = T // 512         # 4 query blocks
KT = T // 128         # 16 key tiles
SCALE = 1.0 / np.sqrt(D)


def build_nc(loops=1, upto='full'):
    nc = bacc.Bacc("TRN2", target_bir_lowering=False, debug=False,
                   num_devices=8)
    _patch_compile(nc)
    xT = nc.dram_tensor("xT", [CI, QB, 128, 512], F32R, kind="ExternalInput")
    Wq = nc.dram_tensor("Wq", [PAIRS, CI, 128, 128], F32R,
                        kind="ExternalInput")
    Wk = nc.dram_tensor("Wk", [PAIRS, CI, 128, 128], F32R,
                        kind="ExternalInput")
    Wv = nc.dram_tensor("Wv", [C, CL], F32R, kind="ExternalInput")
    Wp = nc.dram_tensor("Wp", [CL, C], F32R, kind="ExternalInput")
    bqt = nc.dram_tensor("bqt", [128, PAIRS], F32, kind="ExternalInput")
    bkt = nc.dram_tensor("bkt", [128, PAIRS], F32, kind="ExternalInput")
    bvb = nc.dram_tensor("bvb", [128, CL], F32, kind="ExternalInput")
    ONES = nc.dram_tensor("ONES", [128, 8], F32R, kind="ExternalInput")
    OUT = nc.dram_tensor("OUT", [T, C], F32, kind="ExternalOutput")

    with tile.TileContext(nc) as tc, ExitStack() as ctx:
        sb = ctx.enter_context(tc.tile_pool(name="sb", bufs=1))
        ps = ctx.enter_context(tc.tile_pool(name="ps", bufs=1, space="PSUM"))

        def loop_body():
            # --- constants ---
            bq_sb = sb.tile([128, PAIRS], F32, tag="bq", bufs=1, name="bq_sb")
            nc.sync.dma_start(bq_sb[:], bqt[:])
            bk_sb = sb.tile([128, PAIRS], F32, tag="bk", bufs=1, name="bk_sb")
            nc.sync.dma_start(bk_sb[:], bkt[:])
            bv_sb = sb.tile([128, CL], F32, tag="bv", bufs=1, name="bv_sb")
            nc.sync.dma_start(bv_sb[:], bvb[:])
            ones8 = sb.tile([128, 8], F32R, tag="ones8", bufs=1, name="ones8")
            nc.sync.dma_start(ones8[:], ONES[:])

            # --- V projection: V_ext[kt] = [x @ Wv + bv | 1] resident SBUF ---
            wv_t = []
            for ci in range(CI):
                w = sb.tile([128, CL], F32R, tag="wv", bufs=CI, name=f"wv{ci}")
                nc.sync.dma_start(w[:], Wv[ci * 128:(ci + 1) * 128, :])
                wv_t.append(w)
            v_sb = []
            for ktg in range(KT // 4):
                chunks = []
                for ci in range(CI):
                    c_ = sb.tile([128, 512], F32R, tag="xtc", bufs=14,
                                 name=f"xtc{ci}")
                    nc.sync.dma_start(c_[:], xT[ci, ktg])
                    chunks.append(c_)
                for k2 in range(4):
                    kt = ktg * 4 + k2
                    pv = ps.tile([128, 512], F32, tag="mm", bufs=2, name="pv")
                    for ci in range(CI):
                        nc.tensor.matmul(
                            pv[:], chunks[ci][:, k2 * 128:(k2 + 1) * 128],
                            wv_t[ci][:], start=(ci == 0), stop=(ci == CI - 1))
                    vt = sb.tile([128, HL * 65], F32R, tag="vsb", bufs=KT,
                                 name=f"v{kt}")
                    v3 = vt[:].rearrange("p (h e) -> p h e", e=65)
                    nc.vector.tensor_add(
                        v3[:, :, 0:64],
                        pv[:].rearrange("p (h e) -> p h e", e=64),
                        bv_sb[:].rearrange("p (h e) -> p h e", e=64))
                    nc.vector.tensor_copy(v3[:, :, 64:65], ones8[:])
                    v_sb.append(vt)

            if upto == 'v':
                o_ = sb.tile([128, 512], F32, tag="dbg", bufs=2, name="o_")
                nc.vector.tensor_copy(o_[:], v_sb[-1][:, :512])
                nc.sync.dma_start(OUT[0:128, 0:512], o_[:])
                return

            # --- per head-pair: Q/K projection then attention ---
            ao_t = []
            for j in range(PAIRS):
                wq_t, wk_t = [], []
                for ci in range(CI):
                    wq = sb.tile([128, 128], F32R, tag="wqk", bufs=20,
                                 name=f"wq{ci}")
                    nc.sync.dma_start(wq[:], Wq[j, ci])
                    wq_t.append(wq)
                    wk = sb.tile([128, 128], F32R, tag="wqk", bufs=20,
                                 name=f"wk{ci}")
                    nc.sync.dma_start(wk[:], Wk[j, ci])
                    wk_t.append(wk)
                qt_j = sb.tile([128, T], F32R, tag="qt", bufs=2, name="qt_j")
                kt_j = sb.tile([128, T], F32R, tag="kt", bufs=2, name="kt_j")
                for tb in range(QB):
                    chunks = []
                    for ci in range(CI):
                        c_ = sb.tile([128, 512], F32R, tag="xtc", bufs=14,
                                     name=f"xtc{ci}")
                        nc.sync.dma_start(c_[:], xT[ci, tb])
                        chunks.append(c_)
                    pq = ps.tile([128, 512], F32, tag="mm", bufs=2, name="pq")
                    for ci in range(CI):
                        nc.tensor.matmul(pq[:], wq_t[ci][:], chunks[ci][:],
                                         start=(ci == 0), stop=(ci == CI - 1))
                    nc.vector.tensor_scalar_add(
                        qt_j[:, tb * 512:(tb + 1) * 512], pq[:],
                        bq_sb[:, j:j + 1])
                    pk = ps.tile([128, 512], F32, tag="mm", bufs=2, name="pk")
                    for ci in range(CI):
                        nc.tensor.matmul(pk[:], wk_t[ci][:], chunks[ci][:],
                                         start=(ci == 0), stop=(ci == CI - 1))
                    nc.vector.tensor_scalar_add(
                        kt_j[:, tb * 512:(tb + 1) * 512], pk[:],
                        bk_sb[:, j:j + 1])

                if upto == 'proj':
                    o_ = sb.tile([128, C], F32, tag="dbg2", bufs=2, name="o_")
                    nc.vector.tensor_copy(o_[:], qt_j[:, :C])
                    nc.sync.dma_start(OUT[j * 128:(j + 1) * 128, :], o_[:])
                    continue

                # attention for the two heads of this pair (packed QK)
                ao = sb.tile([128, T], F32R, tag="ao", bufs=PAIRS,
                             name=f"ao{j}")
                ao_t.append(ao)

                G = KT // 2
                po_by_qb = {}

                def emit_qk_exp(qb, g):
                    # interleave the two heads' matmuls so row-group packing
                    # keeps both PE array halves busy concurrently
                    pss2 = []
                    for h in range(2):
                        pss2.append(ps.tile([128, 1024], F32, tag="s",
                                            bufs=2, name="pss"))
                    for k2 in range(2):
                        kt = g * 2 + k2
                        for h in range(2):
                            r0 = h * 64
                            nc.tensor.matmul(
                                pss2[h][:, k2 * 512:(k2 + 1) * 512],
                                kt_j[r0:r0 + 64, kt * 128:(kt + 1) * 128],
                                qt_j[r0:r0 + 64, qb * 512:(qb + 1) * 512],
                                start=True, stop=True)
                    pt2 = [None, None]
                    for h in range(2):
                        pt = sb.tile([128, 1024], F32R, tag="pt", bufs=8,
                                     name="pt")
                        nc.scalar.activation(
                            pt[:], pss2[h][:],
                            mybir.ActivationFunctionType.Exp, scale=SCALE)
                        pt2[h] = pt
                    return pt2

                def emit_av(qb, g, pt2):
                    for h in range(2):
                        hh = 2 * j + h
                        if g == 0:
                            po_by_qb.setdefault(qb, [None, None])[h] = \
                                ps.tile([65, 512], F32, tag="o", bufs=2,
                                        name="po")
                        for k2 in range(2):
                            kt = g * 2 + k2
                            nc.tensor.matmul(
                                po_by_qb[qb][h][:],
                                v_sb[kt][:, hh * 65:(hh + 1) * 65],
                                pt2[h][:, k2 * 512:(k2 + 1) * 512],
                                start=(kt == 0), stop=(kt == KT - 1))

                def emit_norm(qb):
                    po = po_by_qb.pop(qb)
                    for h in range(2):
                        recip = sb.tile([1, 512], F32, tag="recip", bufs=2,
                                        name="recip")
                        nc.vector.reciprocal(recip[:], po[h][64:65, :])
                        rb = sb.tile([64, 512], F32, tag="rb", bufs=2,
                                     name="rb")
                        nc.gpsimd.partition_broadcast(rb[:], recip[:])
                        if h == 0:
                            nc.vector.tensor_mul(
                                ao[0:64, qb * 512:(qb + 1) * 512],
                                po[h][0:64, :], rb[:])
                        else:
                            odd = sb.tile([64, 512], F32R, tag="odd", bufs=2,
                                          name="odd")
                            nc.vector.tensor_mul(odd[:], po[h][0:64, :],
                                                 rb[:])
                            nc.sync.dma_start(
                                ao[64:128, qb * 512:(qb + 1) * 512], odd[:])

                if upto == 'qk':
                    # probe: QK matmuls only
                    for qb in range(QB):
                        for g in range(G):
                            pss2 = [ps.tile([128, 1024], F32, tag="s",
                                            bufs=2, name="pss")
                                    for _ in range(2)]
                            for k2 in range(2):
                                kt = g * 2 + k2
                                for h in range(2):
                                    r0 = h * 64
                                    nc.tensor.matmul(
                                        pss2[h][:, k2 * 512:(k2 + 1) * 512],
                                        kt_j[r0:r0 + 64,
                                             kt * 128:(kt + 1) * 128],
                                        qt_j[r0:r0 + 64,
                                             qb * 512:(qb + 1) * 512],
                                        start=True, stop=True)
                    continue
                if upto == 'qkexp':
                    # probe: QK + exp, no AV
                    for qb in range(QB):
                        for g in range(G):
                            emit_qk_exp(qb, g)
                    continue
                # flat (qb, g) stream with AV lagging one group behind QK/exp
                prev = None
                for qb in range(QB):
                    for g in range(G):
                        pt2 = emit_qk_exp(qb, g)
                        if prev is not None:
                            pqb, pg, ppt2 = prev
                            emit_av(pqb, pg, ppt2)
                            if pg == G - 1:
                                emit_norm(pqb)
                        prev = (qb, g, pt2)
                pqb, pg, ppt2 = prev
                emit_av(pqb, pg, ppt2)
                emit_norm(pqb)

            if upto in ('proj', 'qk', 'qkexp'):
                return

            # --- output projection: OUT_partial = att_outT.T @ Wp ---
            # Wp tiles share the wv pool slots (Wv is dead by now).
            wp_t = []
            for cl in range(PAIRS):
                row = []
                for cb in range(2):
                    wp = sb.tile([128, 512], F32R, tag="wv", bufs=CI,
                                 name=f"wp{cl}_{cb}")
                    nc.sync.dma_start(
                        wp[:], Wp[cl * 128:(cl + 1) * 128,
                                  cb * 512:(cb + 1) * 512])
                    row.append(wp)
                wp_t.append(row)
            if upto == 'attn':
                for jj in range(PAIRS):
                    o_ = sb.tile([128, C], F32, tag="dbg2", bufs=2, name="o_")
                    nc.vector.tensor_copy(o_[:], ao_t[jj][:, :C])
                    nc.sync.dma_start(OUT[jj * 128:(jj + 1) * 128, :], o_[:])
                return
            for qt_i in range(TT):
                ou = sb.tile([128, C], F32, tag="ou", bufs=2, name="ou")
                for cb in range(2):
                    pp = ps.tile([128, 512], F32, tag="mm", bufs=2, name="pp")
                    for cl in range(PAIRS):
                        nc.tensor.matmul(
                            pp[:], ao_t[cl][:, qt_i * 128:(qt_i + 1) * 128],
                            wp_t[cl][cb][:],
                            start=(cl == 0), stop=(cl == PAIRS - 1))
                    nc.vector.tensor_copy(ou[:, cb * 512:(cb + 1) * 512],
                                          pp[:])
                nc.sync.dma_start(OUT[qt_i * 128:(qt_i + 1) * 128, :], ou[:])

        if loops == 1:
            loop_body()
        else:
            with tc.For_i(0, loops, 1):
                loop_body()
    nc.compile()
    return nc


_nc_cache = {}


def get_nc(loops=1, upto='full'):
    key = (loops, upto)
    if key not in _nc_cache:
        _nc_cache[key] = build_nc(loops, upto)
    return _nc_cache[key]


def make_in_maps(x, Wq, bq, Wk, bk, Wv, bv, Wp, bp):
    x = np.asarray(x, dtype=np.float32)
    Wq, Wk, Wv, Wp = (np.asarray(w, dtype=np.float32) for w in (Wq, Wk, Wv, Wp))
    bq, bk, bv = (np.asarray(b_, dtype=np.float32) for b_ in (bq, bk, bv))
    in_maps = []
    for core in range(8):
        b = core // 2
        g = core % 2
        cols = slice(g * CL, (g + 1) * CL)
        # xT tiled: [CI, QB, 128, 512]; xT[ci, tb, p, t] = x[b][tb*512+t, ci*128+p]
        xt = np.ascontiguousarray(
            x[b].T.reshape(CI, 128, QB, 512).transpose(0, 2, 1, 3))
        in_maps.append({
            "xT": xt,
            "Wq": np.ascontiguousarray(
                Wq[:, cols].reshape(CI, 128, PAIRS, 128)
                .transpose(2, 0, 1, 3)),
            "Wk": np.ascontiguousarray(
                Wk[:, cols].reshape(CI, 128, PAIRS, 128)
                .transpose(2, 0, 1, 3)),
            "Wv": np.ascontiguousarray(Wv[:, cols]),
            "Wp": np.ascontiguousarray(Wp[g * CL:(g + 1) * CL, :]),
            "bqt": np.ascontiguousarray(bq[cols].reshape(PAIRS, 128).T),
            "bkt": np.ascontiguousarray(bk[cols].reshape(PAIRS, 128).T),
            "bvb": np.broadcast_to(bv[cols], (128, CL)).copy(),
            "ONES": np.ones((128, 8), np.float32),
        })
    return in_maps


def combine(results, bp):
    out = np.empty((B, T, C), np.float32)
    for b in range(B):
        out[b] = results[2 * b]["OUT"] + results[2 * b + 1]["OUT"] + bp
    return out


def kernel(x, Wq, bq, Wk, bk, Wv, bv, Wp, bp):
    nc = get_nc(1)
    in_maps = make_in_maps(x, Wq, bq, Wk, bk, Wv, bv, Wp, bp)
    res = run_bass_kernel_spmd(nc, in_maps, list(range(8)))
    return combine(res.results, np.asarray(bp, dtype=np.float32))


if __name__ == "__main__":
    import time
    t0 = time.time()
    nc = build_nc(1)
    print("build+compile:", time.time() - t0)



# revision 5
# speedup vs baseline: 1.1585x; 1.1585x over previous
"""MultiHeadAttention Trainium2 kernel (B=4, T=2048, C=1024, H=16, D=64).

Sharding: 8 cores = 4 batches x 2 head-groups. Core c handles batch c//2 and
heads (c%2)*8 .. (c%2)*8+7. Each core computes QKV projections for its head
group, attention, and a partial output projection (its head group's rows of
Wp). Host sums the two partials per batch and adds bp.

Device layout notes:
- All matmul operands are float32r (TF32-like, full PE rate at N>=512,
  ~1.5e-4 matmul relative error).
- x arrives from host pre-transposed AND pre-tiled as xT [CI, QB, 128, 512]
  so every [128,512] chunk DMA is fully contiguous.
- Q,K are computed transposed (QT/KT [c_out, T]); scores are computed
  transposed (scoresT [k, q]) so softmax exp runs on ACT and the AV matmul
  (lhsT=V_ext [k,65], rhs=PT [k,q]) directly yields att_outT [65, q] with
  row 64 = softmax denominator via a ones-column in V_ext. V_ext stays
  resident in SBUF.
- QK matmuls for the two heads of a partition-tile pair are packed into PE
  row halves (tile_position via base_partition) for 2x throughput at K=64.
- AV matmuls are interleaved into the QK/exp group loop so the probability
  tiles are consumed as soon as ACT produces them.
"""
import sys
sys.path.insert(0, '/opt/trn_rl_repo')
from contextlib import ExitStack

import numpy as np

import concourse.bass as bass
import concourse.tile as tile
from concourse import mybir, bacc
from concourse.bass_utils import run_bass_kernel_spmd

F32 = mybir.dt.float32
F32R = mybir.dt.float32r
BF16 = mybir.dt.bfloat16
# Matmul-operand dtype: bf16 runs the PE at full rate on HW (fp32r is
# modeled as full-rate by CoreSim but measures ~2x slower on silicon).
MM_DT = BF16


def _patch_compile(nc):
    """Bacc.compile leaves multi-wait instructions that walrus rejects
    (fuse passes re-merge event-semaphore waits after the last
    generate_event_semaphores). Re-split after every compile."""
    import bass_rust
    orig = nc.compile

    def patched():
        orig()
        bass_rust.generate_event_semaphores(nc)

    nc.compile = patched


B, T, C = 4, 2048, 1024
H = 16
D = C // H            # 64
HL = 8                # heads per core
CL = HL * D           # 512 local channels
PAIRS = HL // 2       # head pairs (one 128-partition tile each)
CI = C // 128         # 8 contraction tiles over C
TT = T // 128         # 16 token tiles
QB = T // 512         # 4 query blocks
KT = T // 128         # 16 key tiles
SCALE = 1.0 / np.sqrt(D)


def build_nc(loops=1, upto='full'):
    nc = bacc.Bacc("TRN2", target_bir_lowering=False, debug=False,
                   num_devices=8)
    _patch_compile(nc)
    xT = nc.dram_tensor("xT", [CI, QB, 128, 512], MM_DT, kind="ExternalInput")
    Wq = nc.dram_tensor("Wq", [PAIRS, CI, 128, 128], MM_DT,
                        kind="ExternalInput")
    Wk = nc.dram_tensor("Wk", [PAIRS, CI, 128, 128], MM_DT,
                        kind="ExternalInput")
    Wv = nc.dram_tensor("Wv", [C, CL], MM_DT, kind="ExternalInput")
    Wp = nc.dram_tensor("Wp", [CL, C], MM_DT, kind="ExternalInput")
    bqt = nc.dram_tensor("bqt", [128, PAIRS], F32, kind="ExternalInput")
    bkt = nc.dram_tensor("bkt", [128, PAIRS], F32, kind="ExternalInput")
    bvb = nc.dram_tensor("bvb", [128, CL], F32, kind="ExternalInput")
    ONES = nc.dram_tensor("ONES", [128, 8], MM_DT, kind="ExternalInput")
    OUT = nc.dram_tensor("OUT", [T, C], F32, kind="ExternalOutput")

    with tile.TileContext(nc) as tc, ExitStack() as ctx:
        sb = ctx.enter_context(tc.tile_pool(name="sb", bufs=1))
        ps = ctx.enter_context(tc.tile_pool(name="ps", bufs=1, space="PSUM"))

        def loop_body():
            # --- constants ---
            bq_sb = sb.tile([128, PAIRS], F32, tag="bq", bufs=1, name="bq_sb")
            nc.sync.dma_start(bq_sb[:], bqt[:])
            bk_sb = sb.tile([128, PAIRS], F32, tag="bk", bufs=1, name="bk_sb")
            nc.sync.dma_start(bk_sb[:], bkt[:])
            bv_sb = sb.tile([128, CL], F32, tag="bv", bufs=1, name="bv_sb")
            nc.sync.dma_start(bv_sb[:], bvb[:])
            ones8 = sb.tile([128, 8], MM_DT, tag="ones8", bufs=1, name="ones8")
            nc.sync.dma_start(ones8[:], ONES[:])

            # --- V projection: V_ext[kt] = [x @ Wv + bv | 1] resident SBUF ---
            wv_t = []
            for ci in range(CI):
                w = sb.tile([128, CL], MM_DT, tag="wv", bufs=CI, name=f"wv{ci}")
                nc.sync.dma_start(w[:], Wv[ci * 128:(ci + 1) * 128, :])
                wv_t.append(w)
            v_sb = []
            for ktg in range(KT // 4):
                chunks = []
                for ci in range(CI):
                    c_ = sb.tile([128, 512], MM_DT, tag="xtc", bufs=14,
                                 name=f"xtc{ci}")
                    nc.sync.dma_start(c_[:], xT[ci, ktg])
                    chunks.append(c_)
                for k2 in range(4):
                    kt = ktg * 4 + k2
                    pv = ps.tile([128, 512], F32, tag="mm", bufs=2, name="pv")
                    for ci in range(CI):
                        nc.tensor.matmul(
                            pv[:], chunks[ci][:, k2 * 128:(k2 + 1) * 128],
                            wv_t[ci][:], start=(ci == 0), stop=(ci == CI - 1))
                    vt = sb.tile([128, HL * 65], MM_DT, tag="vsb", bufs=KT,
                                 name=f"v{kt}")
                    v3 = vt[:].rearrange("p (h e) -> p h e", e=65)
                    nc.vector.tensor_add(
                        v3[:, :, 0:64],
                        pv[:].rearrange("p (h e) -> p h e", e=64),
                        bv_sb[:].rearrange("p (h e) -> p h e", e=64))
                    nc.vector.tensor_copy(v3[:, :, 64:65], ones8[:])
                    v_sb.append(vt)

            if upto == 'v':
                o_ = sb.tile([128, 512], F32, tag="dbg", bufs=2, name="o_")
                nc.vector.tensor_copy(o_[:], v_sb[-1][:, :512])
                nc.sync.dma_start(OUT[0:128, 0:512], o_[:])
                return

            # --- per head-pair: Q/K projection then attention ---
            ao_t = []
            for j in range(PAIRS):
                wq_t, wk_t = [], []
                for ci in range(CI):
                    wq = sb.tile([128, 128], MM_DT, tag="wqk", bufs=20,
                                 name=f"wq{ci}")
                    nc.sync.dma_start(wq[:], Wq[j, ci])
                    wq_t.append(wq)
                    wk = sb.tile([128, 128], MM_DT, tag="wqk", bufs=20,
                                 name=f"wk{ci}")
                    nc.sync.dma_start(wk[:], Wk[j, ci])
                    wk_t.append(wk)
                qt_j = sb.tile([128, T], MM_DT, tag="qt", bufs=2, name="qt_j")
                kt_j = sb.tile([128, T], MM_DT, tag="kt", bufs=2, name="kt_j")
                for tb in range(QB):
                    chunks = []
                    for ci in range(CI):
                        c_ = sb.tile([128, 512], MM_DT, tag="xtc", bufs=14,
                                     name=f"xtc{ci}")
                        nc.sync.dma_start(c_[:], xT[ci, tb])
                        chunks.append(c_)
                    pq = ps.tile([128, 512], F32, tag="mm", bufs=2, name="pq")
                    for ci in range(CI):
                        nc.tensor.matmul(pq[:], wq_t[ci][:], chunks[ci][:],
                                         start=(ci == 0), stop=(ci == CI - 1))
                    nc.vector.tensor_scalar_add(
                        qt_j[:, tb * 512:(tb + 1) * 512], pq[:],
                        bq_sb[:, j:j + 1])
                    pk = ps.tile([128, 512], F32, tag="mm", bufs=2, name="pk")
                    for ci in range(CI):
                        nc.tensor.matmul(pk[:], wk_t[ci][:], chunks[ci][:],
                                         start=(ci == 0), stop=(ci == CI - 1))
                    nc.vector.tensor_scalar_add(
                        kt_j[:, tb * 512:(tb + 1) * 512], pk[:],
                        bk_sb[:, j:j + 1])

                if upto == 'proj':
                    o_ = sb.tile([128, C], F32, tag="dbg2", bufs=2, name="o_")
                    nc.vector.tensor_copy(o_[:], qt_j[:, :C])
                    nc.sync.dma_start(OUT[j * 128:(j + 1) * 128, :], o_[:])
                    continue

                # attention for the two heads of this pair (packed QK)
                ao = sb.tile([128, T], MM_DT, tag="ao", bufs=PAIRS,
                             name=f"ao{j}")
                ao_t.append(ao)

                G = KT // 2
                po_by_qb = {}

                def emit_qk_exp(qb, g):
                    # interleave the two heads' matmuls so row-group packing
                    # keeps both PE array halves busy concurrently
                    pss2 = []
                    for h in range(2):
                        pss2.append(ps.tile([128, 1024], F32, tag="s",
                                            bufs=2, name="pss"))
                    for k2 in range(2):
                        kt = g * 2 + k2
                        for h in range(2):
                            r0 = h * 64
                            nc.tensor.matmul(
                                pss2[h][:, k2 * 512:(k2 + 1) * 512],
                                kt_j[r0:r0 + 64, kt * 128:(kt + 1) * 128],
                                qt_j[r0:r0 + 64, qb * 512:(qb + 1) * 512],
                                start=True, stop=True)
                    pt2 = [None, None]
                    for h in range(2):
                        pt = sb.tile([128, 1024], MM_DT, tag="pt", bufs=8,
                                     name="pt")
                        nc.scalar.activation(
                            pt[:], pss2[h][:],
                            mybir.ActivationFunctionType.Exp, scale=SCALE)
                        pt2[h] = pt
                    return pt2

                def emit_av(qb, g, pt2):
                    for h in range(2):
                        hh = 2 * j + h
                        if g == 0:
                            po_by_qb.setdefault(qb, [None, None])[h] = \
                                ps.tile([65, 512], F32, tag="o", bufs=2,
                                        name="po")
                        for k2 in range(2):
                            kt = g * 2 + k2
                            nc.tensor.matmul(
                                po_by_qb[qb][h][:],
                                v_sb[kt][:, hh * 65:(hh + 1) * 65],
                                pt2[h][:, k2 * 512:(k2 + 1) * 512],
                                start=(kt == 0), stop=(kt == KT - 1))

                def emit_norm(qb):
                    po = po_by_qb.pop(qb)
                    for h in range(2):
                        recip = sb.tile([1, 512], F32, tag="recip", bufs=2,
                                        name="recip")
                        nc.vector.reciprocal(recip[:], po[h][64:65, :])
                        rb = sb.tile([64, 512], F32, tag="rb", bufs=2,
                                     name="rb")
                        nc.gpsimd.partition_broadcast(rb[:], recip[:])
                        if h == 0:
                            nc.vector.tensor_mul(
                                ao[0:64, qb * 512:(qb + 1) * 512],
                                po[h][0:64, :], rb[:])
                        else:
                            odd = sb.tile([64, 512], MM_DT, tag="odd", bufs=2,
                                          name="odd")
                            nc.vector.tensor_mul(odd[:], po[h][0:64, :],
                                                 rb[:])
                            nc.sync.dma_start(
                                ao[64:128, qb * 512:(qb + 1) * 512], odd[:])

                if upto == 'qk':
                    # probe: QK matmuls only
                    for qb in range(QB):
                        for g in range(G):
                            pss2 = [ps.tile([128, 1024], F32, tag="s",
                                            bufs=2, name="pss")
                                    for _ in range(2)]
                            for k2 in range(2):
                                kt = g * 2 + k2
                                for h in range(2):
                                    r0 = h * 64
                                    nc.tensor.matmul(
                                        pss2[h][:, k2 * 512:(k2 + 1) * 512],
                                        kt_j[r0:r0 + 64,
                                             kt * 128:(kt + 1) * 128],
                                        qt_j[r0:r0 + 64,
                                             qb * 512:(qb + 1) * 512],
                                        start=True, stop=True)
                    continue
                if upto == 'qkexp':
                    # probe: QK + exp, no AV
                    for qb in range(QB):
                        for g in range(G):
                            emit_qk_exp(qb, g)
                    continue
                # flat (qb, g) stream with AV lagging one group behind QK/exp
                prev = None
                for qb in range(QB):
                    for g in range(G):
                        pt2 = emit_qk_exp(qb, g)
                        if prev is not None:
                            pqb, pg, ppt2 = prev
                            emit_av(pqb, pg, ppt2)
                            if pg == G - 1:
                                emit_norm(pqb)
                        prev = (qb, g, pt2)
                pqb, pg, ppt2 = prev
                emit_av(pqb, pg, ppt2)
                emit_norm(pqb)

            if upto in ('proj', 'qk', 'qkexp'):
                return

            # --- output projection: OUT_partial = att_outT.T @ Wp ---
            # Wp tiles share the wv pool slots (Wv is dead by now).
            wp_t = []
            for cl in range(PAIRS):
                row = []
                for cb in range(2):
                    wp = sb.tile([128, 512], MM_DT, tag="wv", bufs=CI,
                                 name=f"wp{cl}_{cb}")
                    nc.sync.dma_start(
                        wp[:], Wp[cl * 128:(cl + 1) * 128,
                                  cb * 512:(cb + 1) * 512])
                    row.append(wp)
                wp_t.append(row)
            if upto == 'attn':
                for jj in range(PAIRS):
                    o_ = sb.tile([128, C], F32, tag="dbg2", bufs=2, name="o_")
                    nc.vector.tensor_copy(o_[:], ao_t[jj][:, :C])
                    nc.sync.dma_start(OUT[jj * 128:(jj + 1) * 128, :], o_[:])
                return
            for qt_i in range(TT):
                ou = sb.tile([128, C], F32, tag="ou", bufs=2, name="ou")
                for cb in range(2):
                    pp = ps.tile([128, 512], F32, tag="mm", bufs=2, name="pp")
                    for cl in range(PAIRS):
                        nc.tensor.matmul(
                            pp[:], ao_t[cl][:, qt_i * 128:(qt_i + 1) * 128],
                            wp_t[cl][cb][:],
                            start=(cl == 0), stop=(cl == PAIRS - 1))
                    nc.vector.tensor_copy(ou[:, cb * 512:(cb + 1) * 512],
                                          pp[:])
                nc.sync.dma_start(OUT[qt_i * 128:(qt_i + 1) * 128, :], ou[:])

        if loops == 1:
            loop_body()
        else:
            with tc.For_i(0, loops, 1):
                loop_body()
    nc.compile()
    return nc


_nc_cache = {}


def get_nc(loops=1, upto='full'):
    key = (loops, upto)
    if key not in _nc_cache:
        _nc_cache[key] = build_nc(loops, upto)
    return _nc_cache[key]


def make_in_maps(x, Wq, bq, Wk, bk, Wv, bv, Wp, bp):
    import ml_dtypes
    mm_np = (ml_dtypes.bfloat16 if MM_DT == BF16 else np.float32)
    x = np.asarray(x, dtype=np.float32)
    Wq, Wk, Wv, Wp = (np.asarray(w, dtype=np.float32) for w in (Wq, Wk, Wv, Wp))
    bq, bk, bv = (np.asarray(b_, dtype=np.float32) for b_ in (bq, bk, bv))
    in_maps = []
    for core in range(8):
        b = core // 2
        g = core % 2
        cols = slice(g * CL, (g + 1) * CL)
        # xT tiled: [CI, QB, 128, 512]; xT[ci, tb, p, t] = x[b][tb*512+t, ci*128+p]
        xt = np.ascontiguousarray(
            x[b].T.reshape(CI, 128, QB, 512).transpose(0, 2, 1, 3)
            .astype(mm_np))
        in_maps.append({
            "xT": xt,
            "Wq": np.ascontiguousarray(
                Wq[:, cols].reshape(CI, 128, PAIRS, 128)
                .transpose(2, 0, 1, 3).astype(mm_np)),
            "Wk": np.ascontiguousarray(
                Wk[:, cols].reshape(CI, 128, PAIRS, 128)
                .transpose(2, 0, 1, 3).astype(mm_np)),
            "Wv": np.ascontiguousarray(Wv[:, cols].astype(mm_np)),
            "Wp": np.ascontiguousarray(Wp[g * CL:(g + 1) * CL, :].astype(mm_np)),
            "bqt": np.ascontiguousarray(bq[cols].reshape(PAIRS, 128).T),
            "bkt": np.ascontiguousarray(bk[cols].reshape(PAIRS, 128).T),
            "bvb": np.broadcast_to(bv[cols], (128, CL)).copy(),
            "ONES": np.ones((128, 8), mm_np),
        })
    return in_maps


def combine(results, bp):
    out = np.empty((B, T, C), np.float32)
    for b in range(B):
        out[b] = results[2 * b]["OUT"] + results[2 * b + 1]["OUT"] + bp
    return out


def kernel(x, Wq, bq, Wk, bk, Wv, bv, Wp, bp):
    nc = get_nc(1)
    in_maps = make_in_maps(x, Wq, bq, Wk, bk, Wv, bv, Wp, bp)
    res = run_bass_kernel_spmd(nc, in_maps, list(range(8)))
    return combine(res.results, np.asarray(bp, dtype=np.float32))


if __name__ == "__main__":
    import time
    t0 = time.time()
    nc = build_nc(1)
    print("build+compile:", time.time() - t0)



# revision 8
# speedup vs baseline: 1.3769x; 1.1885x over previous
"""MultiHeadAttention Trainium2 kernel (B=4, T=2048, C=1024, H=16, D=64).

Sharding: 8 cores = 4 batches x 2 head-groups. Core c handles batch c//2 and
heads (c%2)*8 .. (c%2)*8+7. Each core computes QKV projections for its head
group, attention, and a partial output projection (its head group's rows of
Wp). Host sums the two partials per batch and adds bp.

Device layout notes:
- All matmul operands are float32r (TF32-like, full PE rate at N>=512,
  ~1.5e-4 matmul relative error).
- x arrives from host pre-transposed AND pre-tiled as xT [CI, QB, 128, 512]
  so every [128,512] chunk DMA is fully contiguous.
- Q,K are computed transposed (QT/KT [c_out, T]); scores are computed
  transposed (scoresT [k, q]) so softmax exp runs on ACT and the AV matmul
  (lhsT=V_ext [k,65], rhs=PT [k,q]) directly yields att_outT [65, q] with
  row 64 = softmax denominator via a ones-column in V_ext. V_ext stays
  resident in SBUF.
- QK matmuls for the two heads of a partition-tile pair are packed into PE
  row halves (tile_position via base_partition) for 2x throughput at K=64.
- AV matmuls are interleaved into the QK/exp group loop so the probability
  tiles are consumed as soon as ACT produces them.
"""
import sys
sys.path.insert(0, '/opt/trn_rl_repo')
from contextlib import ExitStack

import numpy as np

import concourse.bass as bass
import concourse.tile as tile
from concourse import mybir, bacc
from concourse.bass_utils import run_bass_kernel_spmd

F32 = mybir.dt.float32
F32R = mybir.dt.float32r
BF16 = mybir.dt.bfloat16
# Matmul-operand dtype: bf16 runs the PE at full rate on HW (fp32r is
# modeled as full-rate by CoreSim but measures ~2x slower on silicon).
MM_DT = BF16


def _patch_compile(nc):
    """Bacc.compile leaves multi-wait instructions that walrus rejects
    (fuse passes re-merge event-semaphore waits after the last
    generate_event_semaphores). Re-split after every compile."""
    import bass_rust
    orig = nc.compile

    def patched():
        orig()
        bass_rust.generate_event_semaphores(nc)

    nc.compile = patched


B, T, C = 4, 2048, 1024
H = 16
D = C // H            # 64
HL = 8                # heads per core
CL = HL * D           # 512 local channels
PAIRS = HL // 2       # head pairs (one 128-partition tile each)
CI = C // 128         # 8 contraction tiles over C
TT = T // 128         # 16 token tiles
QB = T // 512         # 4 query blocks
KT = T // 128         # 16 key tiles
SCALE = 1.0 / np.sqrt(D)


def build_nc(loops=1, upto='full'):
    nc = bacc.Bacc("TRN2", target_bir_lowering=False, debug=False,
                   num_devices=8)
    _patch_compile(nc)
    xT = nc.dram_tensor("xT", [CI, QB, 128, 512], MM_DT, kind="ExternalInput")
    Wq = nc.dram_tensor("Wq", [PAIRS, CI, 128, 128], MM_DT,
                        kind="ExternalInput")
    Wk = nc.dram_tensor("Wk", [PAIRS, CI, 128, 128], MM_DT,
                        kind="ExternalInput")
    Wv = nc.dram_tensor("Wv", [C, CL], MM_DT, kind="ExternalInput")
    Wp = nc.dram_tensor("Wp", [CL, C], MM_DT, kind="ExternalInput")
    bqt = nc.dram_tensor("bqt", [128, PAIRS], F32, kind="ExternalInput")
    bkt = nc.dram_tensor("bkt", [128, PAIRS], F32, kind="ExternalInput")
    bvb = nc.dram_tensor("bvb", [128, CL], F32, kind="ExternalInput")
    ONES = nc.dram_tensor("ONES", [128, 8], MM_DT, kind="ExternalInput")
    OUT = nc.dram_tensor("OUT", [T, C], F32, kind="ExternalOutput")

    with tile.TileContext(nc) as tc, ExitStack() as ctx:
        sb = ctx.enter_context(tc.tile_pool(name="sb", bufs=1))
        ps = ctx.enter_context(tc.tile_pool(name="ps", bufs=1, space="PSUM"))

        def loop_body():
            # --- constants ---
            bq_sb = sb.tile([128, PAIRS], F32, tag="bq", bufs=1, name="bq_sb")
            nc.sync.dma_start(bq_sb[:], bqt[:])
            bk_sb = sb.tile([128, PAIRS], F32, tag="bk", bufs=1, name="bk_sb")
            nc.sync.dma_start(bk_sb[:], bkt[:])
            bv_sb = sb.tile([128, CL], F32, tag="bv", bufs=1, name="bv_sb")
            nc.sync.dma_start(bv_sb[:], bvb[:])
            ones8 = sb.tile([128, 8], MM_DT, tag="ones8", bufs=1, name="ones8")
            nc.sync.dma_start(ones8[:], ONES[:])

            # --- V projection: V_ext[kt] = [x @ Wv + bv | 1] resident SBUF ---
            wv_t = []
            for ci in range(CI):
                w = sb.tile([128, CL], MM_DT, tag="wv", bufs=CI, name=f"wv{ci}")
                nc.sync.dma_start(w[:], Wv[ci * 128:(ci + 1) * 128, :])
                wv_t.append(w)
            v_sb = []
            for ktg in range(KT // 4):
                chunks = []
                for ci in range(CI):
                    c_ = sb.tile([128, 512], MM_DT, tag="xtc", bufs=14,
                                 name=f"xtc{ci}")
                    nc.sync.dma_start(c_[:], xT[ci, ktg])
                    chunks.append(c_)
                for k2 in range(4):
                    kt = ktg * 4 + k2
                    pv = ps.tile([128, 512], F32, tag="mm", bufs=4, name="pv")
                    for ci in range(CI):
                        nc.tensor.matmul(
                            pv[:], chunks[ci][:, k2 * 128:(k2 + 1) * 128],
                            wv_t[ci][:], start=(ci == 0), stop=(ci == CI - 1))
                    vt = sb.tile([128, HL * 65], MM_DT, tag="vsb", bufs=KT,
                                 name=f"v{kt}")
                    v3 = vt[:].rearrange("p (h e) -> p h e", e=65)
                    nc.vector.tensor_add(
                        v3[:, :, 0:64],
                        pv[:].rearrange("p (h e) -> p h e", e=64),
                        bv_sb[:].rearrange("p (h e) -> p h e", e=64))
                    nc.vector.tensor_copy(v3[:, :, 64:65], ones8[:])
                    v_sb.append(vt)

            if upto == 'v':
                o_ = sb.tile([128, 512], F32, tag="dbg", bufs=2, name="o_")
                nc.vector.tensor_copy(o_[:], v_sb[-1][:, :512])
                nc.sync.dma_start(OUT[0:128, 0:512], o_[:])
                return

            # --- per head-pair: Q/K projection then attention ---
            ao_t = []
            for j in range(PAIRS):
                wq_t, wk_t = [], []
                for ci in range(CI):
                    wq = sb.tile([128, 128], MM_DT, tag="wqk", bufs=20,
                                 name=f"wq{ci}")
                    nc.sync.dma_start(wq[:], Wq[j, ci])
                    wq_t.append(wq)
                    wk = sb.tile([128, 128], MM_DT, tag="wqk", bufs=20,
                                 name=f"wk{ci}")
                    nc.sync.dma_start(wk[:], Wk[j, ci])
                    wk_t.append(wk)
                qt_j = sb.tile([128, T], MM_DT, tag="qt", bufs=2, name="qt_j")
                kt_j = sb.tile([128, T], MM_DT, tag="kt", bufs=2, name="kt_j")
                for tb in range(QB):
                    chunks = []
                    for ci in range(CI):
                        c_ = sb.tile([128, 512], MM_DT, tag="xtc", bufs=14,
                                     name=f"xtc{ci}")
                        nc.sync.dma_start(c_[:], xT[ci, tb])
                        chunks.append(c_)
                    pq = ps.tile([128, 512], F32, tag="mm", bufs=4, name="pq")
                    for ci in range(CI):
                        nc.tensor.matmul(pq[:], wq_t[ci][:], chunks[ci][:],
                                         start=(ci == 0), stop=(ci == CI - 1))
                    nc.vector.tensor_scalar_add(
                        qt_j[:, tb * 512:(tb + 1) * 512], pq[:],
                        bq_sb[:, j:j + 1])
                    pk = ps.tile([128, 512], F32, tag="mm", bufs=4, name="pk")
                    for ci in range(CI):
                        nc.tensor.matmul(pk[:], wk_t[ci][:], chunks[ci][:],
                                         start=(ci == 0), stop=(ci == CI - 1))
                    nc.vector.tensor_scalar_add(
                        kt_j[:, tb * 512:(tb + 1) * 512], pk[:],
                        bk_sb[:, j:j + 1])

                if upto == 'proj':
                    o_ = sb.tile([128, C], F32, tag="dbg2", bufs=2, name="o_")
                    nc.vector.tensor_copy(o_[:], qt_j[:, :C])
                    nc.sync.dma_start(OUT[j * 128:(j + 1) * 128, :], o_[:])
                    continue

                # attention for the two heads of this pair (packed QK)
                ao = sb.tile([128, T], MM_DT, tag="ao", bufs=PAIRS,
                             name=f"ao{j}")
                ao_t.append(ao)

                G = KT // 2
                po_by_qb = {}

                def emit_qk_exp(qb, g):
                    # interleave the two heads' matmuls so row-group packing
                    # keeps both PE array halves busy concurrently
                    pss2 = []
                    for h in range(2):
                        pss2.append(ps.tile([128, 1024], F32, tag="s",
                                            bufs=2, name="pss"))
                    for k2 in range(2):
                        kt = g * 2 + k2
                        for h in range(2):
                            r0 = h * 64
                            nc.tensor.matmul(
                                pss2[h][:, k2 * 512:(k2 + 1) * 512],
                                kt_j[r0:r0 + 64, kt * 128:(kt + 1) * 128],
                                qt_j[r0:r0 + 64, qb * 512:(qb + 1) * 512],
                                start=True, stop=True)
                    pt2 = [None, None]
                    for h in range(2):
                        pt = sb.tile([128, 1024], MM_DT, tag="pt", bufs=8,
                                     name="pt")
                        nc.scalar.activation(
                            pt[:], pss2[h][:],
                            mybir.ActivationFunctionType.Exp, scale=SCALE)
                        pt2[h] = pt
                    return pt2

                def emit_av(qb, g, pt2):
                    for h in range(2):
                        hh = 2 * j + h
                        if g == 0:
                            po_by_qb.setdefault(qb, [None, None])[h] = \
                                ps.tile([128, 512], F32, tag="mm", bufs=4,
                                        name="po")
                        for k2 in range(2):
                            kt = g * 2 + k2
                            nc.tensor.matmul(
                                po_by_qb[qb][h][0:65, :],
                                v_sb[kt][:, hh * 65:(hh + 1) * 65],
                                pt2[h][:, k2 * 512:(k2 + 1) * 512],
                                start=(kt == 0), stop=(kt == KT - 1))

                def emit_norm(qb):
                    po = po_by_qb.pop(qb)
                    for h in range(2):
                        poS = sb.tile([65, 512], F32, tag="poS", bufs=4,
                                      name="poS")
                        nc.vector.tensor_copy(poS[:], po[h][0:65, :])
                        recip = sb.tile([1, 512], F32, tag="recip", bufs=4,
                                        name="recip")
                        nc.vector.reciprocal(recip[:], poS[64:65, :])
                        rb = sb.tile([64, 512], F32, tag="rb", bufs=4,
                                     name="rb")
                        nc.gpsimd.partition_broadcast(rb[:], recip[:])
                        if h == 0:
                            nc.vector.tensor_mul(
                                ao[0:64, qb * 512:(qb + 1) * 512],
                                poS[0:64, :], rb[:])
                        else:
                            odd = sb.tile([64, 512], MM_DT, tag="odd", bufs=4,
                                          name="odd")
                            nc.vector.tensor_mul(odd[:], poS[0:64, :],
                                                 rb[:])
                            nc.sync.dma_start(
                                ao[64:128, qb * 512:(qb + 1) * 512], odd[:])

                if upto in ('qk', 'qknp'):
                    # probe: QK matmuls only; 'qknp' = no row-half packing
                    for qb in range(QB):
                        for g in range(G):
                            pss2 = [ps.tile([128, 1024], F32, tag="s",
                                            bufs=2, name="pss")
                                    for _ in range(2)]
                            for k2 in range(2):
                                kt = g * 2 + k2
                                for h in range(2):
                                    r0 = h * 64 if upto == 'qk' else 0
                                    nc.tensor.matmul(
                                        pss2[h][:, k2 * 512:(k2 + 1) * 512],
                                        kt_j[r0:r0 + 64,
                                             kt * 128:(kt + 1) * 128],
                                        qt_j[r0:r0 + 64,
                                             qb * 512:(qb + 1) * 512],
                                        start=True, stop=True)
                    continue
                if upto == 'qkexp':
                    # probe: QK + exp, no AV
                    for qb in range(QB):
                        for g in range(G):
                            emit_qk_exp(qb, g)
                    continue
                # flat (qb, g) stream with AV lagging one group behind QK/exp
                prev = None
                for qb in range(QB):
                    for g in range(G):
                        pt2 = emit_qk_exp(qb, g)
                        if prev is not None:
                            pqb, pg, ppt2 = prev
                            emit_av(pqb, pg, ppt2)
                            if pg == G - 1 and upto != 'av':
                                emit_norm(pqb)
                        prev = (qb, g, pt2)
                pqb, pg, ppt2 = prev
                emit_av(pqb, pg, ppt2)
                if upto != 'av':
                    emit_norm(pqb)

            if upto in ('proj', 'qk', 'qknp', 'qkexp', 'av'):
                return

            # --- output projection: OUT_partial = att_outT.T @ Wp ---
            # Wp tiles share the wv pool slots (Wv is dead by now).
            wp_t = []
            for cl in range(PAIRS):
                row = []
                for cb in range(2):
                    wp = sb.tile([128, 512], MM_DT, tag="wv", bufs=CI,
                                 name=f"wp{cl}_{cb}")
                    nc.sync.dma_start(
                        wp[:], Wp[cl * 128:(cl + 1) * 128,
                                  cb * 512:(cb + 1) * 512])
                    row.append(wp)
                wp_t.append(row)
            if upto == 'attn':
                for jj in range(PAIRS):
                    o_ = sb.tile([128, C], F32, tag="dbg2", bufs=2, name="o_")
                    nc.vector.tensor_copy(o_[:], ao_t[jj][:, :C])
                    nc.sync.dma_start(OUT[jj * 128:(jj + 1) * 128, :], o_[:])
                return
            for qt_i in range(TT):
                ou = sb.tile([128, C], F32, tag="ou", bufs=2, name="ou")
                for cb in range(2):
                    pp = ps.tile([128, 512], F32, tag="mm", bufs=4, name="pp")
                    for cl in range(PAIRS):
                        nc.tensor.matmul(
                            pp[:], ao_t[cl][:, qt_i * 128:(qt_i + 1) * 128],
                            wp_t[cl][cb][:],
                            start=(cl == 0), stop=(cl == PAIRS - 1))
                    nc.vector.tensor_copy(ou[:, cb * 512:(cb + 1) * 512],
                                          pp[:])
                nc.sync.dma_start(OUT[qt_i * 128:(qt_i + 1) * 128, :], ou[:])

        if loops == 1:
            loop_body()
        else:
            with tc.For_i(0, loops, 1):
                loop_body()
    nc.compile()
    return nc


_nc_cache = {}


def get_nc(loops=1, upto='full'):
    key = (loops, upto)
    if key not in _nc_cache:
        _nc_cache[key] = build_nc(loops, upto)
    return _nc_cache[key]


def make_in_maps(x, Wq, bq, Wk, bk, Wv, bv, Wp, bp):
    import ml_dtypes
    mm_np = (ml_dtypes.bfloat16 if MM_DT == BF16 else np.float32)
    x = np.asarray(x, dtype=np.float32)
    Wq, Wk, Wv, Wp = (np.asarray(w, dtype=np.float32) for w in (Wq, Wk, Wv, Wp))
    bq, bk, bv = (np.asarray(b_, dtype=np.float32) for b_ in (bq, bk, bv))
    in_maps = []
    for core in range(8):
        b = core // 2
        g = core % 2
        cols = slice(g * CL, (g + 1) * CL)
        # xT tiled: [CI, QB, 128, 512]; xT[ci, tb, p, t] = x[b][tb*512+t, ci*128+p]
        xt = np.ascontiguousarray(
            x[b].T.reshape(CI, 128, QB, 512).transpose(0, 2, 1, 3)
            .astype(mm_np))
        in_maps.append({
            "xT": xt,
            "Wq": np.ascontiguousarray(
                Wq[:, cols].reshape(CI, 128, PAIRS, 128)
                .transpose(2, 0, 1, 3).astype(mm_np)),
            "Wk": np.ascontiguousarray(
                Wk[:, cols].reshape(CI, 128, PAIRS, 128)
                .transpose(2, 0, 1, 3).astype(mm_np)),
            "Wv": np.ascontiguousarray(Wv[:, cols].astype(mm_np)),
            "Wp": np.ascontiguousarray(Wp[g * CL:(g + 1) * CL, :].astype(mm_np)),
            "bqt": np.ascontiguousarray(bq[cols].reshape(PAIRS, 128).T),
            "bkt": np.ascontiguousarray(bk[cols].reshape(PAIRS, 128).T),
            "bvb": np.broadcast_to(bv[cols], (128, CL)).copy(),
            "ONES": np.ones((128, 8), mm_np),
        })
    return in_maps


def combine(results, bp):
    out = np.empty((B, T, C), np.float32)
    for b in range(B):
        out[b] = results[2 * b]["OUT"] + results[2 * b + 1]["OUT"] + bp
    return out


def kernel(x, Wq, bq, Wk, bk, Wv, bv, Wp, bp):
    nc = get_nc(1)
    in_maps = make_in_maps(x, Wq, bq, Wk, bk, Wv, bv, Wp, bp)
    res = run_bass_kernel_spmd(nc, in_maps, list(range(8)))
    return combine(res.results, np.asarray(bp, dtype=np.float32))


if __name__ == "__main__":
    import time
    t0 = time.time()
    nc = build_nc(1)
    print("build+compile:", time.time() - t0)



# revision 11
# speedup vs baseline: 1.3809x; 1.0029x over previous
"""MultiHeadAttention Trainium2 kernel (B=4, T=2048, C=1024, H=16, D=64).

Sharding: 8 cores = 4 batches x 2 head-groups. Core c handles batch c//2 and
heads (c%2)*8 .. (c%2)*8+7. Each core computes QKV projections for its head
group, attention, and a partial output projection (its head group's rows of
Wp). Host sums the two partials per batch and adds bp.

Device layout notes:
- All matmul operands are float32r (TF32-like, full PE rate at N>=512,
  ~1.5e-4 matmul relative error).
- x arrives from host pre-transposed AND pre-tiled as xT [CI, QB, 128, 512]
  so every [128,512] chunk DMA is fully contiguous.
- Q,K are computed transposed (QT/KT [c_out, T]); scores are computed
  transposed (scoresT [k, q]) so softmax exp runs on ACT and the AV matmul
  (lhsT=V_ext [k,65], rhs=PT [k,q]) directly yields att_outT [65, q] with
  row 64 = softmax denominator via a ones-column in V_ext. V_ext stays
  resident in SBUF.
- QK matmuls for the two heads of a partition-tile pair are packed into PE
  row halves (tile_position via base_partition) for 2x throughput at K=64.
- AV matmuls are interleaved into the QK/exp group loop so the probability
  tiles are consumed as soon as ACT produces them.
"""
import sys
sys.path.insert(0, '/opt/trn_rl_repo')
from contextlib import ExitStack

import numpy as np

import concourse.bass as bass
import concourse.tile as tile
from concourse import mybir, bacc
from concourse.bass_utils import run_bass_kernel_spmd

F32 = mybir.dt.float32
F32R = mybir.dt.float32r
BF16 = mybir.dt.bfloat16
# Matmul-operand dtype: bf16 runs the PE at full rate on HW (fp32r is
# modeled as full-rate by CoreSim but measures ~2x slower on silicon).
MM_DT = BF16


def _patch_compile(nc):
    """Bacc.compile leaves multi-wait instructions that walrus rejects
    (fuse passes re-merge event-semaphore waits after the last
    generate_event_semaphores). Re-split after every compile."""
    import bass_rust
    orig = nc.compile

    def patched():
        orig()
        bass_rust.generate_event_semaphores(nc)

    nc.compile = patched


B, T, C = 4, 2048, 1024
H = 16
D = C // H            # 64
HL = 8                # heads per core
CL = HL * D           # 512 local channels
PAIRS = HL // 2       # head pairs (one 128-partition tile each)
CI = C // 128         # 8 contraction tiles over C
TT = T // 128         # 16 token tiles
QB = T // 512         # 4 query blocks
KT = T // 128         # 16 key tiles
SCALE = 1.0 / np.sqrt(D)


def build_nc(loops=1, upto='full'):
    nc = bacc.Bacc("TRN2", target_bir_lowering=False, debug=False,
                   num_devices=8)
    _patch_compile(nc)
    xT = nc.dram_tensor("xT", [CI, 128, T], MM_DT, kind="ExternalInput")
    Wq = nc.dram_tensor("Wq", [PAIRS, 128, CI * 128], MM_DT,
                        kind="ExternalInput")
    Wk = nc.dram_tensor("Wk", [PAIRS, 128, CI * 128], MM_DT,
                        kind="ExternalInput")
    Wv = nc.dram_tensor("Wv", [C, CL], MM_DT, kind="ExternalInput")
    Wp = nc.dram_tensor("Wp", [CL, C], MM_DT, kind="ExternalInput")
    bqt = nc.dram_tensor("bqt", [128, PAIRS], F32, kind="ExternalInput")
    bkt = nc.dram_tensor("bkt", [128, PAIRS], F32, kind="ExternalInput")
    bvb = nc.dram_tensor("bvb", [128, CL], F32, kind="ExternalInput")
    ONES = nc.dram_tensor("ONES", [128, 8], MM_DT, kind="ExternalInput")
    OUT = nc.dram_tensor("OUT", [T, C], F32, kind="ExternalOutput")

    with tile.TileContext(nc) as tc, ExitStack() as ctx:
        sb = ctx.enter_context(tc.tile_pool(name="sb", bufs=1))
        ps = ctx.enter_context(tc.tile_pool(name="ps", bufs=1, space="PSUM"))

        def loop_body():
            # --- constants ---
            bq_sb = sb.tile([128, PAIRS], F32, tag="bq", bufs=1, name="bq_sb")
            nc.sync.dma_start(bq_sb[:], bqt[:])
            bk_sb = sb.tile([128, PAIRS], F32, tag="bk", bufs=1, name="bk_sb")
            nc.sync.dma_start(bk_sb[:], bkt[:])
            bv_sb = sb.tile([128, CL], F32, tag="bv", bufs=1, name="bv_sb")
            nc.sync.dma_start(bv_sb[:], bvb[:])
            ones8 = sb.tile([128, 8], MM_DT, tag="ones8", bufs=1, name="ones8")
            nc.sync.dma_start(ones8[:], ONES[:])

            # --- V projection: V_ext[kt] = [x @ Wv + bv | 1] resident SBUF ---
            wv_t = []
            for ci in range(CI):
                w = sb.tile([128, CL], MM_DT, tag="wv", bufs=CI, name=f"wv{ci}")
                nc.sync.dma_start(w[:], Wv[ci * 128:(ci + 1) * 128, :])
                wv_t.append(w)
            xr = []
            for ci in range(CI):
                xc = sb.tile([128, T], MM_DT, tag="xr", bufs=CI,
                             name=f"xr{ci}")
                nc.sync.dma_start(xc[:], xT[ci])
                xr.append(xc)
            v_sb = []
            for kt in range(KT):
                    pv = ps.tile([128, 512], F32, tag="mm", bufs=4, name="pv")
                    for ci in range(CI):
                        nc.tensor.matmul(
                            pv[:], xr[ci][:, kt * 128:(kt + 1) * 128],
                            wv_t[ci][:], start=(ci == 0), stop=(ci == CI - 1))
                    vt = sb.tile([128, HL * 65], MM_DT, tag="vsb", bufs=KT,
                                 name=f"v{kt}")
                    v3 = vt[:].rearrange("p (h e) -> p h e", e=65)
                    nc.vector.tensor_add(
                        v3[:, :, 0:64],
                        pv[:].rearrange("p (h e) -> p h e", e=64),
                        bv_sb[:].rearrange("p (h e) -> p h e", e=64))
                    nc.vector.tensor_copy(v3[:, :, 64:65], ones8[:])
                    v_sb.append(vt)

            if upto == 'v':
                o_ = sb.tile([128, 512], F32, tag="dbg", bufs=2, name="o_")
                nc.vector.tensor_copy(o_[:], v_sb[-1][:, :512])
                nc.sync.dma_start(OUT[0:128, 0:512], o_[:])
                return

            # --- per head-pair: Q/K projection then attention ---
            ao_t = []
            for j in range(PAIRS):
                wq_j = sb.tile([128, CI * 128], MM_DT, tag="wqk", bufs=4,
                               name="wq_j")
                nc.sync.dma_start(wq_j[:], Wq[j])
                wk_j = sb.tile([128, CI * 128], MM_DT, tag="wqk", bufs=4,
                               name="wk_j")
                nc.sync.dma_start(wk_j[:], Wk[j])
                qt_j = sb.tile([128, T], MM_DT, tag="qt", bufs=2, name="qt_j")
                kt_j = sb.tile([128, T], MM_DT, tag="kt", bufs=2, name="kt_j")
                for tb in range(QB):
                    pq = ps.tile([128, 512], F32, tag="mm", bufs=4, name="pq")
                    for ci in range(CI):
                        nc.tensor.matmul(
                            pq[:], wq_j[:, ci * 128:(ci + 1) * 128],
                            xr[ci][:, tb * 512:(tb + 1) * 512],
                            start=(ci == 0), stop=(ci == CI - 1))
                    nc.vector.tensor_scalar_add(
                        qt_j[:, tb * 512:(tb + 1) * 512], pq[:],
                        bq_sb[:, j:j + 1])
                    pk = ps.tile([128, 512], F32, tag="mm", bufs=4, name="pk")
                    for ci in range(CI):
                        nc.tensor.matmul(
                            pk[:], wk_j[:, ci * 128:(ci + 1) * 128],
                            xr[ci][:, tb * 512:(tb + 1) * 512],
                            start=(ci == 0), stop=(ci == CI - 1))
                    nc.vector.tensor_scalar_add(
                        kt_j[:, tb * 512:(tb + 1) * 512], pk[:],
                        bk_sb[:, j:j + 1])

                if upto == 'proj':
                    o_ = sb.tile([128, C], F32, tag="dbg2", bufs=2, name="o_")
                    nc.vector.tensor_copy(o_[:], qt_j[:, :C])
                    nc.sync.dma_start(OUT[j * 128:(j + 1) * 128, :], o_[:])
                    continue

                # attention for the two heads of this pair (packed QK)
                ao = sb.tile([128, T], MM_DT, tag="ao", bufs=PAIRS,
                             name=f"ao{j}")
                ao_t.append(ao)

                G = KT // 2
                po_by_qb = {}

                def emit_qk_exp(qb, g):
                    # interleave the two heads' matmuls so row-group packing
                    # keeps both PE array halves busy concurrently
                    pss2 = []
                    for h in range(2):
                        pss2.append(ps.tile([128, 1024], F32, tag="s",
                                            bufs=2, name="pss"))
                    for k2 in range(2):
                        kt = g * 2 + k2
                        for h in range(2):
                            r0 = h * 64
                            nc.tensor.matmul(
                                pss2[h][:, k2 * 512:(k2 + 1) * 512],
                                kt_j[r0:r0 + 64, kt * 128:(kt + 1) * 128],
                                qt_j[r0:r0 + 64, qb * 512:(qb + 1) * 512],
                                start=True, stop=True)
                    pt2 = [None, None]
                    for h in range(2):
                        pt = sb.tile([128, 1024], MM_DT, tag="pt", bufs=8,
                                     name="pt")
                        nc.scalar.activation(
                            pt[:], pss2[h][:],
                            mybir.ActivationFunctionType.Exp, scale=SCALE)
                        pt2[h] = pt
                    return pt2

                def emit_av(qb, g, pt2):
                    for h in range(2):
                        hh = 2 * j + h
                        if g == 0:
                            po_by_qb.setdefault(qb, [None, None])[h] = \
                                ps.tile([128, 512], F32, tag="mm", bufs=4,
                                        name="po")
                        for k2 in range(2):
                            kt = g * 2 + k2
                            nc.tensor.matmul(
                                po_by_qb[qb][h][0:65, :],
                                v_sb[kt][:, hh * 65:(hh + 1) * 65],
                                pt2[h][:, k2 * 512:(k2 + 1) * 512],
                                start=(kt == 0), stop=(kt == KT - 1))

                def emit_norm(qb):
                    po = po_by_qb.pop(qb)
                    for h in range(2):
                        poS = sb.tile([65, 512], F32, tag="poS", bufs=4,
                                      name="poS")
                        nc.vector.tensor_copy(poS[:], po[h][0:65, :])
                        recip = sb.tile([1, 512], F32, tag="recip", bufs=4,
                                        name="recip")
                        nc.vector.reciprocal(recip[:], poS[64:65, :])
                        rb = sb.tile([64, 512], F32, tag="rb", bufs=4,
                                     name="rb")
                        nc.gpsimd.partition_broadcast(rb[:], recip[:])
                        if h == 0:
                            nc.vector.tensor_mul(
                                ao[0:64, qb * 512:(qb + 1) * 512],
                                poS[0:64, :], rb[:])
                        else:
                            odd = sb.tile([64, 512], MM_DT, tag="odd", bufs=4,
                                          name="odd")
                            nc.vector.tensor_mul(odd[:], poS[0:64, :],
                                                 rb[:])
                            nc.sync.dma_start(
                                ao[64:128, qb * 512:(qb + 1) * 512], odd[:])

                if upto in ('qk', 'qknp'):
                    # probe: QK matmuls only; 'qknp' = no row-half packing
                    for qb in range(QB):
                        for g in range(G):
                            pss2 = [ps.tile([128, 1024], F32, tag="s",
                                            bufs=2, name="pss")
                                    for _ in range(2)]
                            for k2 in range(2):
                                kt = g * 2 + k2
                                for h in range(2):
                                    r0 = h * 64 if upto == 'qk' else 0
                                    nc.tensor.matmul(
                                        pss2[h][:, k2 * 512:(k2 + 1) * 512],
                                        kt_j[r0:r0 + 64,
                                             kt * 128:(kt + 1) * 128],
                                        qt_j[r0:r0 + 64,
                                             qb * 512:(qb + 1) * 512],
                                        start=True, stop=True)
                    continue
                if upto == 'qkexp':
                    # probe: QK + exp, no AV
                    for qb in range(QB):
                        for g in range(G):
                            emit_qk_exp(qb, g)
                    continue
                # flat (qb, g) stream with AV lagging one group behind QK/exp
                prev = None
                for qb in range(QB):
                    for g in range(G):
                        pt2 = emit_qk_exp(qb, g)
                        if prev is not None:
                            pqb, pg, ppt2 = prev
                            emit_av(pqb, pg, ppt2)
                            if pg == G - 1 and upto != 'av':
                                emit_norm(pqb)
                        prev = (qb, g, pt2)
                pqb, pg, ppt2 = prev
                emit_av(pqb, pg, ppt2)
                if upto != 'av':
                    emit_norm(pqb)

            if upto in ('proj', 'qk', 'qknp', 'qkexp', 'av'):
                return

            # --- output projection: OUT_partial = att_outT.T @ Wp ---
            # Wp tiles share the wv pool slots (Wv is dead by now).
            wp_t = []
            for cl in range(PAIRS):
                wp = sb.tile([128, C], MM_DT, tag="wp", bufs=PAIRS,
                             name=f"wp{cl}")
                nc.sync.dma_start(wp[:], Wp[cl * 128:(cl + 1) * 128, :])
                wp_t.append(wp)

            if upto == 'attn':
                for jj in range(PAIRS):
                    o_ = sb.tile([128, C], F32, tag="dbg2", bufs=2, name="o_")
                    nc.vector.tensor_copy(o_[:], ao_t[jj][:, :C])
                    nc.sync.dma_start(OUT[jj * 128:(jj + 1) * 128, :], o_[:])
                return
            for qt_i in range(TT):
                ou = sb.tile([128, C], F32, tag="ou", bufs=2, name="ou")
                for cb in range(2):
                    pp = ps.tile([128, 512], F32, tag="mm", bufs=4, name="pp")
                    for cl in range(PAIRS):
                        nc.tensor.matmul(
                            pp[:], ao_t[cl][:, qt_i * 128:(qt_i + 1) * 128],
                            wp_t[cl][:, cb * 512:(cb + 1) * 512],
                            start=(cl == 0), stop=(cl == PAIRS - 1))
                    nc.vector.tensor_copy(ou[:, cb * 512:(cb + 1) * 512],
                                          pp[:])
                nc.sync.dma_start(OUT[qt_i * 128:(qt_i + 1) * 128, :], ou[:])

        if loops == 1:
            loop_body()
        else:
            with tc.For_i(0, loops, 1):
                loop_body()
    nc.compile()
    return nc


_nc_cache = {}


def get_nc(loops=1, upto='full'):
    key = (loops, upto)
    if key not in _nc_cache:
        _nc_cache[key] = build_nc(loops, upto)
    return _nc_cache[key]


def make_in_maps(x, Wq, bq, Wk, bk, Wv, bv, Wp, bp):
    import ml_dtypes
    mm_np = (ml_dtypes.bfloat16 if MM_DT == BF16 else np.float32)
    x = np.asarray(x, dtype=np.float32)
    Wq, Wk, Wv, Wp = (np.asarray(w, dtype=np.float32) for w in (Wq, Wk, Wv, Wp))
    bq, bk, bv = (np.asarray(b_, dtype=np.float32) for b_ in (bq, bk, bv))
    in_maps = []
    for core in range(8):
        b = core // 2
        g = core % 2
        cols = slice(g * CL, (g + 1) * CL)
        # xT tiled: [CI, 128, T]; xT[ci, p, t] = x[b][t, ci*128+p]
        xt = np.ascontiguousarray(
            x[b].T.reshape(CI, 128, T).astype(mm_np))
        # Wq/Wk: [PAIRS, 128, CI*128]; [j, p, ci*128+n] = W[ci*128+p, cols[j*128+n]]
        in_maps.append({
            "xT": xt,
            "Wq": np.ascontiguousarray(
                Wq[:, cols].reshape(CI, 128, PAIRS, 128)
                .transpose(2, 1, 0, 3).reshape(PAIRS, 128, CI * 128)
                .astype(mm_np)),
            "Wk": np.ascontiguousarray(
                Wk[:, cols].reshape(CI, 128, PAIRS, 128)
                .transpose(2, 1, 0, 3).reshape(PAIRS, 128, CI * 128)
                .astype(mm_np)),
            "Wv": np.ascontiguousarray(Wv[:, cols].astype(mm_np)),
            "Wp": np.ascontiguousarray(Wp[g * CL:(g + 1) * CL, :].astype(mm_np)),
            "bqt": np.ascontiguousarray(bq[cols].reshape(PAIRS, 128).T),
            "bkt": np.ascontiguousarray(bk[cols].reshape(PAIRS, 128).T),
            "bvb": np.broadcast_to(bv[cols], (128, CL)).copy(),
            "ONES": np.ones((128, 8), mm_np),
        })
    return in_maps


def combine(results, bp):
    out = np.empty((B, T, C), np.float32)
    for b in range(B):
        out[b] = results[2 * b]["OUT"] + results[2 * b + 1]["OUT"] + bp
    return out


def kernel(x, Wq, bq, Wk, bk, Wv, bv, Wp, bp):
    nc = get_nc(1)
    in_maps = make_in_maps(x, Wq, bq, Wk, bk, Wv, bv, Wp, bp)
    res = run_bass_kernel_spmd(nc, in_maps, list(range(8)))
    return combine(res.results, np.asarray(bp, dtype=np.float32))


if __name__ == "__main__":
    import time
    t0 = time.time()
    nc = build_nc(1)
    print("build+compile:", time.time() - t0)



# revision 15
# speedup vs baseline: 1.5073x; 1.0916x over previous
"""MultiHeadAttention Trainium2 kernel (B=4, T=2048, C=1024, H=16, D=64).

Sharding: 8 cores = 4 batches x 2 head-groups. Core c handles batch c//2 and
heads (c%2)*8 .. (c%2)*8+7. Each core computes QKV projections for its head
group, attention, and a partial output projection (its head group's rows of
Wp). Host sums the two partials per batch and adds bp.

Device layout notes:
- All matmul operands are float32r (TF32-like, full PE rate at N>=512,
  ~1.5e-4 matmul relative error).
- x arrives from host pre-transposed AND pre-tiled as xT [CI, QB, 128, 512]
  so every [128,512] chunk DMA is fully contiguous.
- Q,K are computed transposed (QT/KT [c_out, T]); scores are computed
  transposed (scoresT [k, q]) so softmax exp runs on ACT and the AV matmul
  (lhsT=V_ext [k,65], rhs=PT [k,q]) directly yields att_outT [65, q] with
  row 64 = softmax denominator via a ones-column in V_ext. V_ext stays
  resident in SBUF.
- QK matmuls for the two heads of a partition-tile pair are packed into PE
  row halves (tile_position via base_partition) for 2x throughput at K=64.
- AV matmuls are interleaved into the QK/exp group loop so the probability
  tiles are consumed as soon as ACT produces them.
"""
import sys
sys.path.insert(0, '/opt/trn_rl_repo')
from contextlib import ExitStack

import numpy as np

import concourse.bass as bass
import concourse.tile as tile
from concourse import mybir, bacc
from concourse.bass_utils import run_bass_kernel_spmd

F32 = mybir.dt.float32
F32R = mybir.dt.float32r
BF16 = mybir.dt.bfloat16
# Matmul-operand dtype: bf16 runs the PE at full rate on HW (fp32r is
# modeled as full-rate by CoreSim but measures ~2x slower on silicon).
MM_DT = BF16


def _patch_compile(nc):
    """Bacc.compile leaves multi-wait instructions that walrus rejects
    (fuse passes re-merge event-semaphore waits after the last
    generate_event_semaphores). Re-split after every compile."""
    import bass_rust
    orig = nc.compile

    def patched():
        orig()
        bass_rust.generate_event_semaphores(nc)

    nc.compile = patched


B, T, C = 4, 2048, 1024
H = 16
D = C // H            # 64
HL = 8                # heads per core
CL = HL * D           # 512 local channels
PAIRS = HL // 2       # head pairs (one 128-partition tile each)
CI = C // 128         # 8 contraction tiles over C
TT = T // 128         # 16 token tiles
QB = T // 512         # 4 query blocks
KT = T // 128         # 16 key tiles
SCALE = 1.0 / np.sqrt(D)


def build_nc(loops=1, upto='full'):
    nc = bacc.Bacc("TRN2", target_bir_lowering=False, debug=False,
                   num_devices=8)
    _patch_compile(nc)
    xT = nc.dram_tensor("xT", [CI, 128, T], MM_DT, kind="ExternalInput")
    Wq = nc.dram_tensor("Wq", [PAIRS, 128, CI * 128], MM_DT,
                        kind="ExternalInput")
    Wk = nc.dram_tensor("Wk", [PAIRS, 128, CI * 128], MM_DT,
                        kind="ExternalInput")
    Wv = nc.dram_tensor("Wv", [C, CL], MM_DT, kind="ExternalInput")
    Wp = nc.dram_tensor("Wp", [CL, C], MM_DT, kind="ExternalInput")
    bqt = nc.dram_tensor("bqt", [128, PAIRS], F32, kind="ExternalInput")
    bkt = nc.dram_tensor("bkt", [128, PAIRS], F32, kind="ExternalInput")
    bvb = nc.dram_tensor("bvb", [128, CL], F32, kind="ExternalInput")
    ONES = nc.dram_tensor("ONES", [128, 8], MM_DT, kind="ExternalInput")
    OUT = nc.dram_tensor("OUT", [T, C], F32, kind="ExternalOutput")

    with tile.TileContext(nc) as tc, ExitStack() as ctx:
        sb = ctx.enter_context(tc.tile_pool(name="sb", bufs=1))
        ps = ctx.enter_context(tc.tile_pool(name="ps", bufs=1, space="PSUM"))

        def loop_body():
            # --- constants ---
            bq_sb = sb.tile([128, PAIRS], F32, tag="bq", bufs=1, name="bq_sb")
            nc.sync.dma_start(bq_sb[:], bqt[:])
            bk_sb = sb.tile([128, PAIRS], F32, tag="bk", bufs=1, name="bk_sb")
            nc.sync.dma_start(bk_sb[:], bkt[:])
            bv_sb = sb.tile([128, CL], F32, tag="bv", bufs=1, name="bv_sb")
            nc.sync.dma_start(bv_sb[:], bvb[:])
            ones8 = sb.tile([128, 8], MM_DT, tag="ones8", bufs=1, name="ones8")
            nc.sync.dma_start(ones8[:], ONES[:])

            # --- V projection: V_ext[kt] = [x @ Wv + bv | 1] resident SBUF ---
            wv_t = []
            for ci in range(CI):
                w = sb.tile([128, CL], MM_DT, tag="wv", bufs=CI, name=f"wv{ci}")
                nc.sync.dma_start(w[:], Wv[ci * 128:(ci + 1) * 128, :])
                wv_t.append(w)
            xr = []
            for ci in range(CI):
                xc = sb.tile([128, T], MM_DT, tag="xr", bufs=CI,
                             name=f"xr{ci}")
                nc.sync.dma_start(xc[:], xT[ci])
                xr.append(xc)
            v_sb = []
            for kt in range(KT):
                    pv = ps.tile([128, 512], F32, tag="mm", bufs=2, name="pv")
                    for ci in range(CI):
                        nc.tensor.matmul(
                            pv[:], xr[ci][:, kt * 128:(kt + 1) * 128],
                            wv_t[ci][:], start=(ci == 0), stop=(ci == CI - 1))
                    vt = sb.tile([128, HL * 65], MM_DT, tag="vsb", bufs=KT,
                                 name=f"v{kt}")
                    v3 = vt[:].rearrange("p (h e) -> p h e", e=65)
                    nc.vector.tensor_add(
                        v3[:, :, 0:64],
                        pv[:].rearrange("p (h e) -> p h e", e=64),
                        bv_sb[:].rearrange("p (h e) -> p h e", e=64))
                    nc.vector.tensor_copy(v3[:, :, 64:65], ones8[:])
                    v_sb.append(vt)

            if upto == 'v':
                o_ = sb.tile([128, 512], F32, tag="dbg", bufs=2, name="o_")
                nc.vector.tensor_copy(o_[:], v_sb[-1][:, :512])
                nc.sync.dma_start(OUT[0:128, 0:512], o_[:])
                return

            # --- per head-pair: Q/K projection then attention ---
            ao_t = []
            for j in range(PAIRS):
                wq_j = sb.tile([128, CI * 128], MM_DT, tag="wqk", bufs=4,
                               name="wq_j")
                nc.sync.dma_start(wq_j[:], Wq[j])
                wk_j = sb.tile([128, CI * 128], MM_DT, tag="wqk", bufs=4,
                               name="wk_j")
                nc.sync.dma_start(wk_j[:], Wk[j])
                qt_j = sb.tile([128, T], MM_DT, tag="qt", bufs=2, name="qt_j")
                kt_j = sb.tile([128, T], MM_DT, tag="kt", bufs=2, name="kt_j")
                for tb in range(QB):
                    pq = ps.tile([128, 512], F32, tag="mm", bufs=2, name="pq")
                    for ci in range(CI):
                        nc.tensor.matmul(
                            pq[:], wq_j[:, ci * 128:(ci + 1) * 128],
                            xr[ci][:, tb * 512:(tb + 1) * 512],
                            start=(ci == 0), stop=(ci == CI - 1))
                    nc.vector.tensor_scalar_add(
                        qt_j[:, tb * 512:(tb + 1) * 512], pq[:],
                        bq_sb[:, j:j + 1])
                    pk = ps.tile([128, 512], F32, tag="mm", bufs=2, name="pk")
                    for ci in range(CI):
                        nc.tensor.matmul(
                            pk[:], wk_j[:, ci * 128:(ci + 1) * 128],
                            xr[ci][:, tb * 512:(tb + 1) * 512],
                            start=(ci == 0), stop=(ci == CI - 1))
                    nc.vector.tensor_scalar_add(
                        kt_j[:, tb * 512:(tb + 1) * 512], pk[:],
                        bk_sb[:, j:j + 1])

                if upto == 'proj':
                    o_ = sb.tile([128, C], F32, tag="dbg2", bufs=2, name="o_")
                    nc.vector.tensor_copy(o_[:], qt_j[:, :C])
                    nc.sync.dma_start(OUT[j * 128:(j + 1) * 128, :], o_[:])
                    continue

                # attention for the two heads of this pair (packed QK)
                ao = sb.tile([128, T], MM_DT, tag="ao", bufs=PAIRS,
                             name=f"ao{j}")
                ao_t.append(ao)

                G = KT // 2
                po_by_qb = {}

                def emit_qk_exp(qb, g):
                    # interleave the two heads' matmuls so row-group packing
                    # keeps both PE array halves busy concurrently
                    pss2 = []
                    for h in range(2):
                        pss2.append(ps.tile([128, 1024], F32, tag="s",
                                            bufs=3, name="pss"))
                    for k2 in range(2):
                        kt = g * 2 + k2
                        for h in range(2):
                            r0 = h * 64
                            nc.tensor.matmul(
                                pss2[h][:, k2 * 512:(k2 + 1) * 512],
                                kt_j[r0:r0 + 64, kt * 128:(kt + 1) * 128],
                                qt_j[r0:r0 + 64, qb * 512:(qb + 1) * 512],
                                start=True, stop=True)
                    pt2 = [None, None]
                    for h in range(2):
                        pt = sb.tile([128, 1024], MM_DT, tag="pt", bufs=8,
                                     name="pt")
                        nc.scalar.activation(
                            pt[:], pss2[h][:],
                            mybir.ActivationFunctionType.Exp, scale=SCALE)
                        pt2[h] = pt
                    return pt2

                def emit_av(qb, g, pt2):
                    for h in range(2):
                        hh = 2 * j + h
                        if g == 0:
                            po_by_qb.setdefault(qb, [None, None])[h] = \
                                ps.tile([128, 512], F32, tag="mm", bufs=2,
                                        name="po")
                        for k2 in range(2):
                            kt = g * 2 + k2
                            if upto == 'av2':
                                st, sp = True, True
                            else:
                                st, sp = (kt == 0), (kt == KT - 1)
                            nc.tensor.matmul(
                                po_by_qb[qb][h][0:65, :],
                                v_sb[kt][:, hh * 65:(hh + 1) * 65],
                                pt2[h][:, k2 * 512:(k2 + 1) * 512],
                                start=st, stop=sp)

                def emit_norm(qb):
                    po = po_by_qb.pop(qb)
                    for h in range(2):
                        poS = sb.tile([65, 512], F32, tag="poS", bufs=4,
                                      name="poS")
                        nc.vector.tensor_copy(poS[:], po[h][0:65, :])
                        recip = sb.tile([1, 512], F32, tag="recip", bufs=4,
                                        name="recip")
                        nc.vector.reciprocal(recip[:], poS[64:65, :])
                        rb = sb.tile([64, 512], F32, tag="rb", bufs=4,
                                     name="rb")
                        nc.gpsimd.partition_broadcast(rb[:], recip[:])
                        if h == 0:
                            nc.vector.tensor_mul(
                                ao[0:64, qb * 512:(qb + 1) * 512],
                                poS[0:64, :], rb[:])
                        else:
                            odd = sb.tile([64, 512], MM_DT, tag="odd", bufs=4,
                                          name="odd")
                            nc.vector.tensor_mul(odd[:], poS[0:64, :],
                                                 rb[:])
                            nc.sync.dma_start(
                                ao[64:128, qb * 512:(qb + 1) * 512], odd[:])

                if upto in ('qk', 'qknp'):
                    # probe: QK matmuls only; 'qknp' = no row-half packing
                    for qb in range(QB):
                        for g in range(G):
                            pss2 = [ps.tile([128, 1024], F32, tag="s",
                                            bufs=2, name="pss")
                                    for _ in range(2)]
                            for k2 in range(2):
                                kt = g * 2 + k2
                                for h in range(2):
                                    r0 = h * 64 if upto == 'qk' else 0
                                    nc.tensor.matmul(
                                        pss2[h][:, k2 * 512:(k2 + 1) * 512],
                                        kt_j[r0:r0 + 64,
                                             kt * 128:(kt + 1) * 128],
                                        qt_j[r0:r0 + 64,
                                             qb * 512:(qb + 1) * 512],
                                        start=True, stop=True)
                    continue
                if upto == 'qkexp':
                    # probe: QK + exp, no AV
                    for qb in range(QB):
                        for g in range(G):
                            emit_qk_exp(qb, g)
                    continue
                # flat (qb, g) stream with AV lagging one group behind QK/exp
                probe = upto in ('av', 'av2', 'avpe', 'peonly')
                pt_const = None
                prev = None
                for qb in range(QB):
                    for g in range(G):
                        if upto == 'peonly':
                            # QK matmuls only (no exp), AV reads stale pt
                            pss2 = [ps.tile([128, 1024], F32, tag="s",
                                            bufs=2, name="pss")
                                    for _ in range(2)]
                            for k2 in range(2):
                                kt = g * 2 + k2
                                for h in range(2):
                                    r0 = h * 64
                                    nc.tensor.matmul(
                                        pss2[h][:, k2 * 512:(k2 + 1) * 512],
                                        kt_j[r0:r0 + 64,
                                             kt * 128:(kt + 1) * 128],
                                        qt_j[r0:r0 + 64,
                                             qb * 512:(qb + 1) * 512],
                                        start=True, stop=True)
                            if pt_const is None:
                                pt_const = []
                                for _ in range(2):
                                    ptc = sb.tile([128, 1024], MM_DT,
                                                  tag="pt", bufs=8,
                                                  name="ptc")
                                    nc.vector.tensor_copy(ptc[:],
                                                          kt_j[:, 0:1024])
                                    pt_const.append(ptc)
                            pt2 = pt_const
                        else:
                            pt2 = emit_qk_exp(qb, g)
                            if upto == 'avpe':
                                if pt_const is None:
                                    pt_const = pt2
                                pt2 = pt_const
                        if prev is not None:
                            pqb, pg, ppt2 = prev
                            emit_av(pqb, pg, ppt2)
                            if pg == G - 1 and not probe:
                                emit_norm(pqb)
                        prev = (qb, g, pt2)
                pqb, pg, ppt2 = prev
                emit_av(pqb, pg, ppt2)
                if not probe:
                    emit_norm(pqb)

            if upto in ('proj', 'qk', 'qknp', 'qkexp', 'av', 'av2', 'avpe',
                        'peonly'):
                return

            # --- output projection: OUT_partial = att_outT.T @ Wp ---
            # Wp tiles share the wv pool slots (Wv is dead by now).
            wp_t = []
            for cl in range(PAIRS):
                wp = sb.tile([128, C], MM_DT, tag="wp", bufs=PAIRS,
                             name=f"wp{cl}")
                nc.sync.dma_start(wp[:], Wp[cl * 128:(cl + 1) * 128, :])
                wp_t.append(wp)

            if upto == 'attn':
                for jj in range(PAIRS):
                    o_ = sb.tile([128, C], F32, tag="dbg2", bufs=2, name="o_")
                    nc.vector.tensor_copy(o_[:], ao_t[jj][:, :C])
                    nc.sync.dma_start(OUT[jj * 128:(jj + 1) * 128, :], o_[:])
                return
            for qt_i in range(TT):
                ou = sb.tile([128, C], F32, tag="ou", bufs=2, name="ou")
                for cb in range(2):
                    pp = ps.tile([128, 512], F32, tag="mm", bufs=2, name="pp")
                    for cl in range(PAIRS):
                        nc.tensor.matmul(
                            pp[:], ao_t[cl][:, qt_i * 128:(qt_i + 1) * 128],
                            wp_t[cl][:, cb * 512:(cb + 1) * 512],
                            start=(cl == 0), stop=(cl == PAIRS - 1))
                    nc.vector.tensor_copy(ou[:, cb * 512:(cb + 1) * 512],
                                          pp[:])
                nc.sync.dma_start(OUT[qt_i * 128:(qt_i + 1) * 128, :], ou[:])

        if loops == 1:
            loop_body()
        else:
            with tc.For_i(0, loops, 1):
                loop_body()
    nc.compile()
    return nc


_nc_cache = {}


def get_nc(loops=1, upto='full'):
    key = (loops, upto)
    if key not in _nc_cache:
        _nc_cache[key] = build_nc(loops, upto)
    return _nc_cache[key]


def make_in_maps(x, Wq, bq, Wk, bk, Wv, bv, Wp, bp):
    import ml_dtypes
    mm_np = (ml_dtypes.bfloat16 if MM_DT == BF16 else np.float32)
    x = np.asarray(x, dtype=np.float32)
    Wq, Wk, Wv, Wp = (np.asarray(w, dtype=np.float32) for w in (Wq, Wk, Wv, Wp))
    bq, bk, bv = (np.asarray(b_, dtype=np.float32) for b_ in (bq, bk, bv))
    in_maps = []
    for core in range(8):
        b = core // 2
        g = core % 2
        cols = slice(g * CL, (g + 1) * CL)
        # xT tiled: [CI, 128, T]; xT[ci, p, t] = x[b][t, ci*128+p]
        xt = np.ascontiguousarray(
            x[b].T.reshape(CI, 128, T).astype(mm_np))
        # Wq/Wk: [PAIRS, 128, CI*128]; [j, p, ci*128+n] = W[ci*128+p, cols[j*128+n]]
        in_maps.append({
            "xT": xt,
            "Wq": np.ascontiguousarray(
                Wq[:, cols].reshape(CI, 128, PAIRS, 128)
                .transpose(2, 1, 0, 3).reshape(PAIRS, 128, CI * 128)
                .astype(mm_np)),
            "Wk": np.ascontiguousarray(
                Wk[:, cols].reshape(CI, 128, PAIRS, 128)
                .transpose(2, 1, 0, 3).reshape(PAIRS, 128, CI * 128)
                .astype(mm_np)),
            "Wv": np.ascontiguousarray(Wv[:, cols].astype(mm_np)),
            "Wp": np.ascontiguousarray(Wp[g * CL:(g + 1) * CL, :].astype(mm_np)),
            "bqt": np.ascontiguousarray(bq[cols].reshape(PAIRS, 128).T),
            "bkt": np.ascontiguousarray(bk[cols].reshape(PAIRS, 128).T),
            "bvb": np.broadcast_to(bv[cols], (128, CL)).copy(),
            "ONES": np.ones((128, 8), mm_np),
        })
    return in_maps


def combine(results, bp):
    out = np.empty((B, T, C), np.float32)
    for b in range(B):
        out[b] = results[2 * b]["OUT"] + results[2 * b + 1]["OUT"] + bp
    return out


def kernel(x, Wq, bq, Wk, bk, Wv, bv, Wp, bp):
    nc = get_nc(1)
    in_maps = make_in_maps(x, Wq, bq, Wk, bk, Wv, bv, Wp, bp)
    res = run_bass_kernel_spmd(nc, in_maps, list(range(8)))
    return combine(res.results, np.asarray(bp, dtype=np.float32))


if __name__ == "__main__":
    import time
    t0 = time.time()
    nc = build_nc(1)
    print("build+compile:", time.time() - t0)

